# revision 2
# baseline (speedup 1.0000x reference)
"""Batch-hard triplet loss on 8 Trainium2 NeuronCores, v2.

Key structure (v1 tricks retained: label-sorted batch + per-core column
rotation, static positive window, dict one-hot mask matmul carrying
BIG*[same] + ||x_j||^2 hi/lo rows):

- bf16 everywhere on the PE: measured on HW, fp16 moving operands
  stream at 0.83 ns/col (conversion penalty) while bf16/fp8 stream at
  the full 0.42 ns/col. sq_j rides as bf16 hi+lo rows (~1e-3 abs).
  MAINS_DTYPE=fp16 env flag switches back for an accuracy A/B.
- All mask/dict tables host-built and DMA'd (no on-device builds).
- Three-way PSUM drain, engine-balanced (per-core LP: PE 12.0,
  DVE 13.5, Act 12.0 us):
    * cols [0:1024]    mask matmul folds BIG+norm; Act wide fp32 copy
                       -> SBUF; DVE 2x min + 2x window-max
    * cols [1024:3072] PE norm-fold; Act wide copy; DVE 2x min
    * cols [3072:4096] no PE norm; DVE tensor_tensor_reduce direct
                       from PSUM with SBUF sq_j table (fold+min in one
                       1x pass)
- Single batched final epilogue over [128, MC] tiles.
"""

import os

import numpy as np
import ml_dtypes

import concourse.bass as bass
import concourse.tile as tile
from concourse import bacc, mybir
from concourse.bass_utils import run_bass_kernel_spmd

B = 4096          # batch
D = 128           # embedding dim
NCORES = 8
R = B // NCORES   # rows per core (512)
MC = R // 128     # 128-row chunks per core (4)
NB = 512          # column block (one PSUM bank at fp32)
MB = 1024         # masked band: local columns [0, MB) can hold same-labels
ROLL = 256        # local col j = global (j + c*R - ROLL) mod B
BAND = 192        # max distance row -> same-label column (host-asserted)

BIGC = 2048.0     # same-label offset code (max d2 ~ 477)
TAU = 50.0        # has-positive threshold on max same d2 (min real ~136)
MARGIN = 0.3

F32 = mybir.dt.float32
BF16 = mybir.dt.bfloat16
F16 = mybir.dt.float16
ALU = mybir.AluOpType
ACTF = mybir.ActivationFunctionType
AXX = mybir.AxisListType.X

USE_FP16 = os.environ.get("MAINS_DTYPE", "bf16") == "fp16"
MDT = F16 if USE_FP16 else BF16
NPDT = np.float16 if USE_FP16 else ml_dtypes.bfloat16
# tensor_tensor_reduce compiles but faults at runtime on this HW stack;
# default to PE norm matmuls on banks 6-7 instead.
USE_TTR = bool(os.environ.get("USE_TTR"))

_CACHE: dict = {}


def build_nc() -> bass.Bass:
    nc = bacc.Bacc(None, target_bir_lowering=False)

    xt = nc.declare_dram_parameter("xt", [D, B], MDT, isOutput=False)
    xsn = nc.declare_dram_parameter("xsn", [D, R], MDT, isOutput=False)
    lh = nc.declare_dram_parameter("lh", [128, MC * 128], MDT, isOutput=False)
    rhs = nc.declare_dram_parameter("rhs", [128, MC * MB], MDT,
                                    isOutput=False)
    sqhl = nc.declare_dram_parameter("sqhl", [2, B], MDT, isOutput=False)
    sqb = nc.declare_dram_parameter("sqb", [128, MB], F32, isOutput=False)
    sqs = nc.declare_dram_parameter("sqs", [128, MC], F32, isOutput=False)
    sqsb = nc.declare_dram_parameter("sqsb", [128, MC], F32, isOutput=False)
    out = nc.declare_dram_parameter("out", [128, 2], F32, isOutput=True)

    with tile.TileContext(nc) as tc:
        with (
            tc.tile_pool(name="const", bufs=1) as cpool,
            tc.tile_pool(name="psum", bufs=1, space="PSUM") as psum,
            tc.tile_pool(name="mva", bufs=2) as mva,
            tc.tile_pool(name="mvb", bufs=2) as mvb,
            tc.tile_pool(name="mvt", bufs=2) as mvt,
            tc.tile_pool(name="stats", bufs=1) as stats,
        ):
            # ---- input DMA ----
            # Contiguous per-block tiles: matmuls reading strided slices
            # of big tiles stream at half rate (measured), so XT and RHS
            # land in [128, 512] tiles. First-needed transfers first.
            XSN = cpool.tile([D, R], MDT)
            nc.sync.dma_start(XSN[:], xsn[:])
            XTB = [cpool.tile([D, NB], MDT, name=f"xtb{n}") for n in range(8)]
            RHSB = [cpool.tile([128, NB], MDT, name=f"rhsb{k}")
                    for k in range(2 * MC)]
            LH = cpool.tile([128, MC * 128], MDT)
            SQHL = cpool.tile([2, B], MDT)
            SQS = cpool.tile([128, MC], F32)
            SQSB = cpool.tile([128, MC], F32)
            nc.sync.dma_start(XTB[0][:], xt[:, 0:NB])
            nc.scalar.dma_start(XTB[1][:], xt[:, NB:2 * NB])
            nc.scalar.dma_start(RHSB[0][:], rhs[:, 0:NB])
            nc.gpsimd.dma_start(RHSB[1][:], rhs[:, NB:2 * NB])
            nc.sync.dma_start(LH[:], lh[:])
            nc.sync.dma_start(SQHL[:], sqhl[:])
            for n in range(2, 8):
                eng = (nc.sync, nc.scalar, nc.gpsimd)[n % 3]
                eng.dma_start(XTB[n][:], xt[:, n * NB:(n + 1) * NB])
            for k in range(2, 2 * MC):
                eng = (nc.gpsimd, nc.scalar, nc.sync)[k % 3]
                eng.dma_start(RHSB[k][:], rhs[:, k * NB:(k + 1) * NB])
            nc.sync.dma_start(SQS[:], sqs[:])
            nc.sync.dma_start(SQSB[:], sqsb[:])

            ONESH = cpool.tile([2, 128], MDT)
            nc.vector.memset(ONESH[:], 1.0)
            # Dummy sqrt up front: forces the Act table load (the one
            # covering Copy/Identity/Sqrt/Square) into the DMA-wait
            # window instead of the epilogue.
            DUMS = cpool.tile([1, 2], F32)
            nc.vector.memset(DUMS[:, 0:1], 1.0)
            nc.scalar.sqrt(DUMS[:, 1:2], DUMS[:, 0:1])

            # ---- stat collectors ----
            PM = stats.tile([128, MC], F32)       # window max (BIG + posd2')
            NMIN = stats.tile([128, MC], F32)     # min over negatives (d2')
            NMP = stats.tile([128, 3 * MC], F32)  # per-chunk min parts
            SCR = stats.tile([128, 2048], F32)    # shared dummy wide out

            # Per chunk: TA = one 2-bank tile (banks 0-1, mask+window;
            # its 4 matmuls pay the strided-out half-rate penalty but the
            # window stays a single-op reduce), banks 2-7 = per-bank
            # [128,512] tiles so their matmuls write whole tiles and
            # stream at the full 215 ns cadence (measured: strided PSUM
            # outputs halve the matmul rate).
            # Act converts banks 2-7 to fp16; DVE runs a 2x pair-min tree.
            # The DVE queue is software-pipelined: chunk m's tree is
            # emitted after chunk m+1's TA reduces, so DVE never blocks
            # the next chunk's PSUM reuse.
            def emit_pe(m, TA, PB):
                for n in range(2):
                    nc.tensor.matmul(
                        TA[:, n * NB:(n + 1) * NB],
                        XSN[:, bass.ts(m, 128)], XTB[n][:],
                        start=True, stop=False,
                    )
                for n in range(2):
                    nc.tensor.matmul(
                        TA[:, n * NB:(n + 1) * NB],
                        LH[:, bass.ts(m, 128)], RHSB[2 * m + n][:],
                        start=False, stop=True,
                    )
                for n in range(2, 8):
                    nc.tensor.matmul(
                        PB[n - 2][:], XSN[:, bass.ts(m, 128)], XTB[n][:],
                        start=True, stop=False,
                    )
                    nc.tensor.matmul(
                        PB[n - 2][:], ONESH[0:2, :],
                        SQHL[0:2, bass.ts(n, NB)],
                        start=False, stop=True,
                    )

            def emit_act(m, PB):
                hs = []
                for j in range(6):
                    H = mva.tile([128, NB], F16, name=f"h{m}_{j}")
                    nc.scalar.activation(H[:], PB[j][:], ACTF.Copy)
                    hs.append(H)
                return hs

            def emit_dve_early(m, TA):
                ws = m * 128 + 64    # positive window [ws, ws+512)
                nc.vector.tensor_scalar(
                    SCR[:, 0:1024], TA[:], 0.0, None,
                    op0=ALU.add, op1=ALU.min,
                    accum_out=NMP[:, 3 * m:3 * m + 1],
                )
                nc.vector.tensor_scalar(
                    SCR[:, 0:NB], TA[:, ws:ws + NB], 0.0, None,
                    op0=ALU.add, op1=ALU.max, accum_out=PM[:, m:m + 1],
                )

            def emit_dve_tree(m, hs):
                P1 = mvt.tile([128, NB], F16, name=f"p1_{m}")
                nc.vector.tensor_tensor(P1[:], hs[0][:], hs[1][:], op=ALU.min)
                P2 = mvt.tile([128, NB], F16, name=f"p2_{m}")
                nc.vector.tensor_tensor(P2[:], hs[2][:], hs[3][:], op=ALU.min)
                P3 = mvt.tile([128, NB], F16, name=f"p3_{m}")
                nc.vector.tensor_tensor(P3[:], hs[4][:], hs[5][:], op=ALU.min)
                P4 = mvt.tile([128, NB], F16, name=f"p4_{m}")
                nc.vector.tensor_tensor(P4[:], P1[:], P2[:], op=ALU.min)
                P5 = mvt.tile([128, NB], F16, name=f"p5_{m}")
                nc.vector.tensor_tensor(P5[:], P3[:], P4[:], op=ALU.min)
                nc.vector.tensor_scalar(
                    SCR[:, 0:NB], P5[:], 0.0, None,
                    op0=ALU.add, op1=ALU.min,
                    accum_out=NMP[:, 3 * m + 1:3 * m + 2],
                )
                nc.vector.tensor_tensor(
                    NMIN[:, m:m + 1], NMP[:, 3 * m:3 * m + 1],
                    NMP[:, 3 * m + 1:3 * m + 2], op=ALU.min,
                )

            prev = None
            for m in range(MC):
                TA = psum.tile([128, 1024], F32, tag="ta", name=f"ta{m}")
                PB = [psum.tile([128, NB], F32, tag=f"pb{j}", name=f"pb{m}_{j}")
                      for j in range(6)]
                emit_pe(m, TA, PB)
                hs = emit_act(m, PB)
                emit_dve_early(m, TA)
                if prev is not None:
                    emit_dve_tree(*prev)
                prev = (m, hs)
            emit_dve_tree(*prev)

            # ---- batched final epilogue over [128, MC] ----
            E = stats.tile([128, 8 * MC], F32)
            ep = [E[:, i * MC:(i + 1) * MC] for i in range(8)]
            nc.vector.tensor_tensor(ep[0], PM[:], SQSB[:], op=ALU.add)
            nc.vector.tensor_scalar(ep[1], ep[0], TAU, None,
                                    op0=ALU.is_gt, op1=ALU.bypass)
            nc.vector.tensor_scalar(ep[0], ep[0], 0.0, None,
                                    op0=ALU.max, op1=ALU.bypass)
            nc.vector.tensor_tensor(ep[2], NMIN[:], SQS[:], op=ALU.add)
            nc.vector.tensor_scalar(ep[3], ep[2], BIGC / 2.0, None,
                                    op0=ALU.is_lt, op1=ALU.bypass)
            nc.vector.tensor_scalar(ep[2], ep[2], 0.0, None,
                                    op0=ALU.max, op1=ALU.bypass)
            nc.vector.tensor_tensor(ep[4], ep[1], ep[3], op=ALU.mult)
            nc.scalar.sqrt(ep[5], ep[0])
            nc.scalar.sqrt(ep[6], ep[2])
            nc.vector.tensor_tensor(ep[7], ep[5], ep[6], op=ALU.subtract)
            nc.vector.tensor_scalar(ep[7], ep[7], MARGIN, 0.0,
                                    op0=ALU.add, op1=ALU.max)
            nc.vector.tensor_tensor(ep[7], ep[7], ep[4], op=ALU.mult)

            OUT = stats.tile([128, 2], F32)
            nc.vector.tensor_reduce(OUT[:, 0:1], ep[7], axis=AXX, op=ALU.add)
            nc.vector.tensor_reduce(OUT[:, 1:2], ep[4], axis=AXX, op=ALU.add)
            nc.sync.dma_start(out[:], OUT[:])

    nc.compile()
    return nc


def _get_nc() -> bass.Bass:
    if "nc" not in _CACHE:
        _CACHE["nc"] = build_nc()
    return _CACHE["nc"]


def prep_inputs(embeddings: np.ndarray, labels: np.ndarray) -> list[dict]:
    x = np.ascontiguousarray(np.asarray(embeddings, dtype=np.float32))
    lab0 = np.asarray(labels)

    perm = np.argsort(lab0, kind="stable")
    xs = x[perm]
    lab = lab0[perm].astype(np.int64)

    # Host-side guarantee for the static positive window.
    first: dict = {}
    last: dict = {}
    for i, v in enumerate(lab):
        if v not in first:
            first[v] = i
        last[v] = i
    idx = np.arange(B)
    firsts = np.array([first[v] for v in lab])
    lasts = np.array([last[v] for v in lab])
    assert (idx - firsts).max() <= BAND and (lasts - idx).max() <= BAND, \
        "label runs exceed the static positive window"

    xT = np.ascontiguousarray(xs.T)                      # [D, B] f32
    sq64 = np.einsum("ij,ij->i", xs.astype(np.float64), xs.astype(np.float64))
    sqh = sq64.astype(NPDT)
    sql = (sq64 - sqh.astype(np.float64)).astype(NPDT)
    sqhl_g = np.stack([sqh, sql])                        # [2, B]
    sqf = sq64.astype(np.float32)

    slots = np.r_[0:96, 98:128]                          # 126 dict slots

    in_maps = []
    for c in range(NCORES):
        rows = slice(c * R, (c + 1) * R)
        lab_sh = lab[rows]
        roll = ROLL - c * R
        lab_loc = np.roll(lab, roll)                     # local col labels
        sq_loc = np.roll(sqf, roll)
        xt_c = np.ascontiguousarray(np.roll(xT, roll, axis=1).astype(NPDT))
        sqhl_c = np.ascontiguousarray(np.roll(sqhl_g, roll, axis=1))
        xsn_c = np.ascontiguousarray((-2.0 * xT[:, rows]).astype(NPDT))
        sqs_c = np.ascontiguousarray(sqf[rows].reshape(MC, 128).T)
        sqsb_c = np.ascontiguousarray(
            (sqf[rows] - np.float32(BIGC)).reshape(MC, 128).T)
        # sq_j broadcast for the TTR region (local cols 3072:4096)
        sqb_c = np.ascontiguousarray(
            np.broadcast_to(sq_loc[3072:4096], (128, MB)).astype(np.float32))

        lh_c = np.zeros((128, MC * 128), NPDT)
        rhs_c = np.zeros((128, MC * MB), NPDT)
        for m in range(MC):
            lab_m = lab_sh[m * 128:(m + 1) * 128]
            u = np.unique(lab_m)
            assert len(u) <= 126, f"chunk has {len(u)} distinct labels"
            dict_rows = slots[:len(u)]
            lh_m = np.zeros((128, 128), np.float32)
            lh_m[dict_rows, :] = (u[:, None] == lab_m[None, :]) * BIGC
            lh_m[96:98, :] = 1.0
            lh_c[:, m * 128:(m + 1) * 128] = lh_m.astype(NPDT)
            rhs_m = np.zeros((128, MB), np.float32)
            rhs_m[dict_rows, :] = (u[:, None] == lab_loc[None, 0:MB])
            rhs_c[:, m * MB:(m + 1) * MB] = rhs_m.astype(NPDT)
            rhs_c[96:98, m * MB:(m + 1) * MB] = sqhl_c[:, 0:MB]
        in_maps.append({
            "xt": xt_c, "xsn": xsn_c, "lh": lh_c, "rhs": rhs_c,
            "sqhl": sqhl_c, "sqb": sqb_c, "sqs": sqs_c, "sqsb": sqsb_c,
        })
    return in_maps


def combine_outputs(results: list[dict]) -> np.ndarray:
    loss_sum = 0.0
    n_valid = 0.0
    for r in results:
        o = np.asarray(r["out"], dtype=np.float64)
        loss_sum += o[:, 0].sum()
        n_valid += o[:, 1].sum()
    if n_valid > 0:
        val = loss_sum / max(n_valid, 1.0)
    else:
        val = 0.0
    return np.array(val, dtype=np.float32)


def run(embeddings: np.ndarray, labels: np.ndarray, **spmd_kwargs):
    nc = _get_nc()
    in_maps = prep_inputs(embeddings, labels)
    res = run_bass_kernel_spmd(nc, in_maps, core_ids=list(range(NCORES)),
                               **spmd_kwargs)
    return combine_outputs(res.results), res


def kernel(embeddings: np.ndarray, labels: np.ndarray) -> np.ndarray:
    loss, _ = run(embeddings, labels)
    return loss
